# revision 1
# baseline (speedup 1.0000x reference)
"""Dice coefficient metric kernel for TRN2 (8 NeuronCores, SPMD batch-parallel).

Reference computation (all fp32):
    inter[b,c] = sum_hw prd*tgt
    union[b,c] = sum_hw prd + sum_hw tgt + EPS
    dice[b,c]  = (2*inter + EPS) / union
    out[c]     = mean_b dice[b,c]

Sharding: batch dim (16) split across 8 cores -> 2 batches (8 (b,c) slabs
of 1024x1024) per core.  Slabs stream HBM->SBUF as [128, 4096] half-slab
tiles (prd on the SP HWDGE ring, tgt on the ACT ring, 4-deep buffering so
DMA never starves) and are reduced on the DVE with two fused
scalar_tensor_tensor ops per tile (product+accum -> inter partial,
sum+accum -> union partial); the last slab is split into four 2048-wide
quarters so the post-DMA drain is one quarter's compute.  Per-partition
partials land in a [128, 36] stats tile; one ones-vector matmul collapses
the partition dim into PSUM, a handful of tiny DVE ops fold the partials
and form dice, and each core DMAs its per-core dice sum (4 floats) out.
The host sums the 8 partials and divides by B (the batch mean) while
gathering.  Measured: ~182us HW exec vs a ~187us nominal HBM roofline
(64 MiB/core at 358 GB/s).

The device-side AllReduce variant (USE_COLLECTIVE=True) is kept for
reference but off by default: on this runtime a 16-byte 8-core AllReduce
measures ~98us of fixed latency (half the kernel's runtime), and HWDGE DMA
deadlocks when a collective is present in the NEFF, forcing slower SWDGE
loads on top.  tensor_tensor_reduce crashes the exec unit on this runtime;
scalar_tensor_tensor expresses the same fused multiply/add + reduction.
"""

import numpy as np

import concourse.bass as bass
import concourse.tile as tile
from concourse import bacc, mybir
from concourse.bass_utils import run_bass_kernel_spmd

B, C, H, W = 16, 4, 1024, 1024
N_CORES = 8
P = 128
EPS = 1e-6

B_LOC = B // N_CORES          # batches per core
SLABS = B_LOC * C             # (b,c) slabs per core
F = (H * W) // P              # free dim per full slab

USE_COLLECTIVE = False


def _build_nc(slabs: int, feat: int, c: int, n_cores: int):
    """Build + compile the per-core Bass program (same program on all cores)."""
    nc = bacc.Bacc(
        "TRN2", target_bir_lowering=False, debug=False, num_devices=n_cores
    )
    f32 = mybir.dt.float32
    quarter = feat // 4
    prd = nc.dram_tensor("prd", [slabs, P, feat], f32, kind="ExternalInput")
    tgt = nc.dram_tensor("tgt", [slabs, P, feat], f32, kind="ExternalInput")
    out = nc.dram_tensor("out", [1, c], f32, kind="ExternalOutput")

    add = mybir.AluOpType.add
    mult = mybir.AluOpType.mult

    # Without a collective in the NEFF the HWDGE rings (sync/scalar) are
    # safe and faster than SWDGE; with one they deadlock -> use gpsimd.
    load_p = nc.gpsimd if USE_COLLECTIVE else nc.sync
    load_t = nc.gpsimd if USE_COLLECTIVE else nc.scalar

    # (slab, col_offset, width, fold_group, fold_idx) load/reduce units:
    # slabs 0..slabs-2 in halves, the last slab in quarters.  Group a holds
    # {h0 of each full slab, q0, q2}, group b holds {h1, q1, q3}; summing
    # group a + group b columns in one add folds everything pairwise.
    half = feat // 2
    units = []
    for s in range(slabs - 1):
        units.append((s, 0, half, 0, s))
        units.append((s, half, half, 1, s))
    for q in range(4):
        units.append((slabs - 1, q * quarter, quarter, q % 2, slabs - 1 + q // 2))
    n_fold = slabs + 1  # columns per (group, kind)

    with tile.TileContext(nc) as tc:
        with (
            tc.tile_pool(name="io", bufs=4) as io_pool,
            tc.tile_pool(name="work", bufs=1) as work_pool,
            tc.tile_pool(name="psum", bufs=1, space=bass.MemorySpace.PSUM) as psum_pool,
            tc.tile_pool(name="dram", bufs=1, space=bass.MemorySpace.DRAM) as dram_pool,
        ):
            # stats layout: [inter_a | union_a | inter_b | union_b], each
            # n_fold wide; unit (group g, idx i): inter col 2*n_fold*g + i,
            # union col 2*n_fold*g + n_fold + i.  Both reductions run on the
            # DVE as fused scalar_tensor_tensor ops (one pass each); keeping
            # the ACT engine free of datapath work matters because it issues
            # the tgt DMAs (HWDGE) -- ACT compute in the stream delays those
            # issues and starves the DMA (measured +31us).
            stats = work_pool.tile([P, 4 * n_fold], f32)
            scratch = work_pool.tile([P, half], f32)

            for s, off, width, g, i in units:
                pt = io_pool.tile([P, width], f32, tag="prd")
                load_p.dma_start(pt[:], prd[s, :, off : off + width])
                tt = io_pool.tile([P, width], f32, tag="tgt")
                load_t.dma_start(tt[:], tgt[s, :, off : off + width])

                base = 2 * n_fold * g
                # accum_out = sum((pt * 1) op1 tt)
                nc.vector.scalar_tensor_tensor(
                    out=scratch[:, 0:width], in0=pt[:], scalar=1.0, in1=tt[:],
                    op0=mult, op1=mult,
                    accum_out=stats[:, base + i : base + i + 1],
                )
                nc.vector.scalar_tensor_tensor(
                    out=scratch[:, 0:width], in0=pt[:], scalar=1.0, in1=tt[:],
                    op0=mult, op1=add,
                    accum_out=stats[:, base + n_fold + i : base + n_fold + i + 1],
                )

            # Collapse the 128 partitions: ps[0, :] = ones.T @ stats (PSUM).
            ones = work_pool.tile([P, 1], f32)
            nc.vector.memset(ones[:], 1.0)
            ps = psum_pool.tile([1, 4 * n_fold], f32)
            nc.tensor.matmul(ps[:], ones[:], stats[:], start=True, stop=True)

            # One add folds group a + group b (via an SBUF bounce for group b:
            # a DVE op may read at most one input from PSUM); the two extra
            # columns per kind (last slab's quarter pairs) compact after.
            fin = work_pool.tile([1, 2 * n_fold + c], f32)
            gb = work_pool.tile([1, 2 * n_fold], f32)
            nc.vector.tensor_copy(gb[:], ps[0:1, 2 * n_fold : 4 * n_fold])
            nc.vector.tensor_add(
                fin[0:1, 0 : 2 * n_fold],
                ps[0:1, 0 : 2 * n_fold],
                gb[:],
            )
            ls = slabs - 1
            nc.vector.tensor_add(
                fin[0:1, ls : ls + 1],
                fin[0:1, ls : ls + 1],
                fin[0:1, ls + 1 : ls + 2],
            )
            nc.vector.tensor_add(
                fin[0:1, n_fold + ls : n_fold + ls + 1],
                fin[0:1, n_fold + ls : n_fold + ls + 1],
                fin[0:1, n_fold + ls + 1 : n_fold + ls + 2],
            )

            inter = fin[0:1, 0:slabs]
            usum = fin[0:1, n_fold : n_fold + slabs]
            num = work_pool.tile([1, slabs], f32)
            nc.vector.tensor_scalar(num[:], inter, 2.0, EPS, mult, add)
            den = work_pool.tile([1, slabs], f32)
            nc.vector.tensor_scalar(den[:], usum, EPS, None, add)
            rec = work_pool.tile([1, slabs], f32)
            nc.vector.reciprocal(rec[:], den[:])
            dice = work_pool.tile([1, slabs], f32)
            nc.vector.tensor_mul(dice[:], num[:], rec[:])

            # Per-core partial: sum of this core's B_LOC batches per channel
            # (slab s = b_local*C + ch).
            part = fin[0:1, 2 * n_fold : 2 * n_fold + c]
            nc.vector.tensor_add(part, dice[0:1, 0:c], dice[0:1, c : 2 * c])

            if USE_COLLECTIVE:
                cc_in = dram_pool.tile([1, c], f32)
                cc_out = dram_pool.tile([1, c], f32)
                nc.gpsimd.dma_start(cc_in[:], part)
                nc.gpsimd.collective_compute(
                    "AllReduce",
                    add,
                    replica_groups=[list(range(n_cores))],
                    ins=[cc_in.opt()],
                    outs=[cc_out.opt()],
                )
                res = work_pool.tile([1, c], f32)
                nc.gpsimd.dma_start(res[:], cc_out[:])
                nc.vector.tensor_scalar_mul(
                    res[:], res[:], 1.0 / (B_LOC * n_cores)
                )
                nc.gpsimd.dma_start(out[0:1, :], res[:])
            else:
                nc.sync.dma_start(out[0:1, :], part)

    nc.compile()
    return nc


_NC_CACHE: dict = {}


def _get_nc():
    key = (SLABS, F, C, N_CORES)
    if key not in _NC_CACHE:
        _NC_CACHE[key] = _build_nc(*key)
    return _NC_CACHE[key]


def _shard_inputs(prd: np.ndarray, tgt: np.ndarray):
    in_maps = []
    for i in range(N_CORES):
        sl = slice(i * B_LOC, (i + 1) * B_LOC)
        in_maps.append(
            {
                "prd": np.ascontiguousarray(prd[sl]).reshape(SLABS, P, F),
                "tgt": np.ascontiguousarray(tgt[sl]).reshape(SLABS, P, F),
            }
        )
    return in_maps


def kernel(prd: np.ndarray, tgt: np.ndarray, _trace: bool = False):
    prd = np.asarray(prd, dtype=np.float32)
    tgt = np.asarray(tgt, dtype=np.float32)
    assert prd.shape == (B, C, H, W) and tgt.shape == (B, C, H, W)

    nc = _get_nc()
    in_maps = _shard_inputs(prd, tgt)
    res = run_bass_kernel_spmd(nc, in_maps, list(range(N_CORES)), trace=_trace)
    if USE_COLLECTIVE:
        out = res.results[0]["out"].reshape(C).astype(np.float32)
    else:
        out = (
            sum(r["out"].reshape(C).astype(np.float64) for r in res.results) / B
        ).astype(np.float32)
    if _trace:
        return out, res
    return out



# revision 2
# speedup vs baseline: 1.0415x; 1.0415x over previous
"""Dice coefficient metric kernel for TRN2 (8 NeuronCores, SPMD batch-parallel).

Reference computation (all fp32):
    inter[b,c] = sum_hw prd*tgt
    union[b,c] = sum_hw prd + sum_hw tgt + EPS
    dice[b,c]  = (2*inter + EPS) / union
    out[c]     = mean_b dice[b,c]

Sharding: batch dim (16) split across 8 cores -> 2 batches (8 (b,c) slabs
of 1024x1024) per core.  All slabs stream HBM->SBUF on the single SP
HWDGE ring as [128, 4096] half-slab tiles (4-deep buffering); the last
slab is split into four 2048-wide quarters so the post-DMA drain is one
quarter's compute.

Compute is split across engines so no engine comes close to the DMA
floor (the v1 kernel ran both fused reductions on the DVE: 145us busy vs
a ~187us per-core DMA floor under HBM-stack contention, so any bandwidth
dip turned into buffer-recycle stalls and the slowest core landed at
~222us):
  - DVE: one fused scalar_tensor_tensor per tile (prd*tgt product with
    accum -> inter partial), ~73us total.
  - ACT: two activation(Copy, accum_out) ops per tile (sum prd, sum tgt
    -> union partials), ~120us total.  ACT does no DMA issue (an ACT
    compute op in front of a DMA issue delays it and starves the ring,
    measured +31us in v1), which is why all loads sit on the SP ring.
  - PE: two tiny ones-vector matmuls collapse the 128 partitions into
    PSUM; a handful of small DVE ops fold partials and form dice.
Each core DMAs its per-core dice sum (4 floats) out; the host sums the
8 partials and divides by B while gathering.

The device-side AllReduce variant (USE_COLLECTIVE=True) is kept for
reference but off by default: on this runtime a 16-byte 8-core AllReduce
measures ~98us of fixed latency (half the kernel's runtime), and HWDGE
DMA deadlocks when a collective is present in the NEFF, forcing slower
SWDGE loads on top.  tensor_tensor_reduce crashes the exec unit on this
runtime; scalar_tensor_tensor expresses the same fused multiply +
reduction.
"""

import numpy as np

import concourse.bass as bass
import concourse.tile as tile
from concourse import bacc, mybir
from concourse.bass_utils import run_bass_kernel_spmd

B, C, H, W = 16, 4, 1024, 1024
N_CORES = 8
P = 128
EPS = 1e-6

B_LOC = B // N_CORES          # batches per core
SLABS = B_LOC * C             # (b,c) slabs per core
F = (H * W) // P              # free dim per full slab

USE_COLLECTIVE = False


def _build_nc(slabs: int, feat: int, c: int, n_cores: int):
    """Build + compile the per-core Bass program (same program on all cores)."""
    nc = bacc.Bacc(
        "TRN2", target_bir_lowering=False, debug=False, num_devices=n_cores
    )
    f32 = mybir.dt.float32
    quarter = feat // 4
    prd = nc.dram_tensor("prd", [slabs, P, feat], f32, kind="ExternalInput")
    tgt = nc.dram_tensor("tgt", [slabs, P, feat], f32, kind="ExternalInput")
    out = nc.dram_tensor("out", [1, c], f32, kind="ExternalOutput")

    add = mybir.AluOpType.add
    mult = mybir.AluOpType.mult
    copy_fn = mybir.ActivationFunctionType.Copy

    # Without a collective in the NEFF the SP HWDGE ring is safe and
    # faster than SWDGE; with one it deadlocks -> use gpsimd.
    load = nc.gpsimd if USE_COLLECTIVE else nc.sync

    # (slab, col_offset, width, fold_group, fold_idx) load/reduce units:
    # slabs 0..slabs-2 in halves, the last slab in quarters.  Group a holds
    # {h0 of each full slab, q0, q2}, group b holds {h1, q1, q3}; summing
    # group a + group b columns in one add folds everything pairwise.
    half = feat // 2
    units = []
    for s in range(slabs - 1):
        units.append((s, 0, half, 0, s))
        units.append((s, half, half, 1, s))
    for q in range(4):
        units.append((slabs - 1, q * quarter, quarter, q % 2, slabs - 1 + q // 2))
    n_fold = slabs + 1  # columns per (group, kind)

    with tile.TileContext(nc) as tc:
        with (
            tc.tile_pool(name="io", bufs=4) as io_pool,
            tc.tile_pool(name="work", bufs=1) as work_pool,
            tc.tile_pool(name="psum", bufs=1, space=bass.MemorySpace.PSUM) as psum_pool,
            tc.tile_pool(name="dram", bufs=1, space=bass.MemorySpace.DRAM) as dram_pool,
        ):
            # Per-partition partials.  DVE and ACT write separate stats
            # tiles (sharing one would cross-serialize their queues);
            # each collapses with its own ones-vector matmul.
            # stats_int: [int_a | int_b], stats_sum: [pt_a | pt_b | tt_a
            # | tt_b], each group n_fold wide; unit (group g, idx i)
            # lands in column g*n_fold + i of its kind.
            stats_int = work_pool.tile([P, 2 * n_fold], f32)
            stats_sum = work_pool.tile([P, 4 * n_fold], f32)
            dve_scr = work_pool.tile([P, half], f32)
            act_scr = work_pool.tile([P, half], f32)

            for s, off, width, g, i in units:
                pt = io_pool.tile([P, width], f32, tag="prd")
                load.dma_start(pt[:], prd[s, :, off : off + width])
                tt = io_pool.tile([P, width], f32, tag="tgt")
                load.dma_start(tt[:], tgt[s, :, off : off + width])

                col = g * n_fold + i
                # DVE: inter partial = sum((pt * 1) * tt)
                nc.vector.scalar_tensor_tensor(
                    out=dve_scr[:, 0:width], in0=pt[:], scalar=1.0, in1=tt[:],
                    op0=mult, op1=mult,
                    accum_out=stats_int[:, col : col + 1],
                )
                # ACT: union partials = sum(pt), sum(tt)
                nc.scalar.activation(
                    out=act_scr[:, 0:width], in_=pt[:], func=copy_fn,
                    accum_out=stats_sum[:, col : col + 1],
                )
                nc.scalar.activation(
                    out=act_scr[:, 0:width], in_=tt[:], func=copy_fn,
                    accum_out=stats_sum[:, 2 * n_fold + col : 2 * n_fold + col + 1],
                )

            # Collapse the 128 partitions: ps[0, :] = ones.T @ stats (PSUM).
            ones = work_pool.tile([P, 1], f32)
            nc.vector.memset(ones[:], 1.0)
            ps_int = psum_pool.tile([1, 2 * n_fold], f32)
            nc.tensor.matmul(ps_int[:], ones[:], stats_int[:], start=True, stop=True)
            ps_sum = psum_pool.tile([1, 4 * n_fold], f32)
            nc.tensor.matmul(ps_sum[:], ones[:], stats_sum[:], start=True, stop=True)

            # Fold group a + group b (via an SBUF bounce for one side: a
            # DVE op may read at most one input from PSUM), then compact
            # the last slab's quarter pair (columns ls, ls+1).
            ls = slabs - 1
            gb = work_pool.tile([1, n_fold], f32)
            inter = work_pool.tile([1, n_fold], f32)
            nc.vector.tensor_copy(gb[:], ps_int[0:1, n_fold : 2 * n_fold])
            nc.vector.tensor_add(inter[:], ps_int[0:1, 0:n_fold], gb[:])
            nc.vector.tensor_add(
                inter[0:1, ls : ls + 1],
                inter[0:1, ls : ls + 1],
                inter[0:1, ls + 1 : ls + 2],
            )

            pb = work_pool.tile([1, n_fold], f32)
            psm = work_pool.tile([1, n_fold], f32)
            nc.vector.tensor_copy(pb[:], ps_sum[0:1, n_fold : 2 * n_fold])
            nc.vector.tensor_add(psm[:], ps_sum[0:1, 0:n_fold], pb[:])
            tb = work_pool.tile([1, n_fold], f32)
            tsm = work_pool.tile([1, n_fold], f32)
            nc.vector.tensor_copy(tb[:], ps_sum[0:1, 3 * n_fold : 4 * n_fold])
            nc.vector.tensor_add(tsm[:], ps_sum[0:1, 2 * n_fold : 3 * n_fold], tb[:])
            usum = work_pool.tile([1, n_fold], f32)
            nc.vector.tensor_add(usum[:], psm[:], tsm[:])
            nc.vector.tensor_add(
                usum[0:1, ls : ls + 1],
                usum[0:1, ls : ls + 1],
                usum[0:1, ls + 1 : ls + 2],
            )

            num = work_pool.tile([1, slabs], f32)
            nc.vector.tensor_scalar(num[:], inter[0:1, 0:slabs], 2.0, EPS, mult, add)
            den = work_pool.tile([1, slabs], f32)
            nc.vector.tensor_scalar(den[:], usum[0:1, 0:slabs], EPS, None, add)
            rec = work_pool.tile([1, slabs], f32)
            nc.vector.reciprocal(rec[:], den[:])
            dice = work_pool.tile([1, slabs], f32)
            nc.vector.tensor_mul(dice[:], num[:], rec[:])

            # Per-core partial: sum of this core's B_LOC batches per channel
            # (slab s = b_local*C + ch).
            part = work_pool.tile([1, c], f32)
            nc.vector.tensor_add(part[:], dice[0:1, 0:c], dice[0:1, c : 2 * c])

            if USE_COLLECTIVE:
                cc_in = dram_pool.tile([1, c], f32)
                cc_out = dram_pool.tile([1, c], f32)
                nc.gpsimd.dma_start(cc_in[:], part[:])
                nc.gpsimd.collective_compute(
                    "AllReduce",
                    add,
                    replica_groups=[list(range(n_cores))],
                    ins=[cc_in.opt()],
                    outs=[cc_out.opt()],
                )
                res = work_pool.tile([1, c], f32)
                nc.gpsimd.dma_start(res[:], cc_out[:])
                nc.vector.tensor_scalar_mul(
                    res[:], res[:], 1.0 / (B_LOC * n_cores)
                )
                nc.gpsimd.dma_start(out[0:1, :], res[:])
            else:
                nc.sync.dma_start(out[0:1, :], part[:])

    nc.compile()
    return nc


_NC_CACHE: dict = {}


def _get_nc():
    key = (SLABS, F, C, N_CORES)
    if key not in _NC_CACHE:
        _NC_CACHE[key] = _build_nc(*key)
    return _NC_CACHE[key]


def _shard_inputs(prd: np.ndarray, tgt: np.ndarray):
    in_maps = []
    for i in range(N_CORES):
        sl = slice(i * B_LOC, (i + 1) * B_LOC)
        in_maps.append(
            {
                "prd": np.ascontiguousarray(prd[sl]).reshape(SLABS, P, F),
                "tgt": np.ascontiguousarray(tgt[sl]).reshape(SLABS, P, F),
            }
        )
    return in_maps


def kernel(prd: np.ndarray, tgt: np.ndarray, _trace: bool = False):
    prd = np.asarray(prd, dtype=np.float32)
    tgt = np.asarray(tgt, dtype=np.float32)
    assert prd.shape == (B, C, H, W) and tgt.shape == (B, C, H, W)

    nc = _get_nc()
    in_maps = _shard_inputs(prd, tgt)
    res = run_bass_kernel_spmd(nc, in_maps, list(range(N_CORES)), trace=_trace)
    if USE_COLLECTIVE:
        out = res.results[0]["out"].reshape(C).astype(np.float32)
    else:
        out = (
            sum(r["out"].reshape(C).astype(np.float64) for r in res.results) / B
        ).astype(np.float32)
    if _trace:
        return out, res
    return out
